# revision 38
# baseline (speedup 1.0000x reference)
"""CondTransport kernel v4 for 8x Trainium2 NeuronCores.

Math (per reference):
  x_mean = [x_mu, y_mean+y_var]                      [Nq, 64]
  x_var  = [x_mu, 0.01*flip(y_eta), y_mean+y_var]    [Nq, 96]
  Lam_m  = kXXmean_inv @ Z_mean                      [Nx, 32]
  Lam_v  = kXXvar_inv  @ Z_var                       [Nx, 32]
  K_m    = exp(-d2(X_mean, x_mean)/128);  z_m = K_m.T @ Lam_m
  K_v    = exp(-d2(X_var,  x_var )/128);  z_v = K_v.T @ Lam_v
  out    = y_mean + y_var + z_m + z_v                [Nq, 32]

Design (v5, 276us vs 418us v2 baseline):
  * all matmul operands bf16 (host-cast); inv-Gram stream is 32MB/core
    and S matmuls stream at 1 col/cycle (f32r measured ~1.55x slower).
  * ScalarE queue carries ONLY the exp ACTs (the ~145us serial floor);
    all DMAs go to sync (inv stream) / gpsimd (everything else).
  * producer/consumer decoupling: S+exp tiles produced 1/slot from slot
    4; z blocks (4 tiles, 4-way column-tiled into pz4 bands via
    tile_position) consume >=6 tiles behind and >=2 windows behind the
    Lambda gather, so the in-order PE never blocks on ACT/collectives.
  * Lambda stream in 8 windows (4 groups x 2 matrices); chunk DMAs
    prefetch PF slots ahead; Lambda matmuls 4-way column-tiled.
  * export chain keeps Vector minimal (the Tile scheduler statically
    orders each queue with a cost model blind to DMA contention, and
    hoisted comm-input writers block the queue): DVE psum->sbuf copy,
    one PE matmul per 128-col block vs a stacked identity (fuses
    transpose + 4-band reduce into natural layout), DVE copy, DMA,
    AllGather -- all under tile_wait_until hints.
  * d2 trick: S~ = X.q - |X|^2/2 - |q|^2/2 via two extra contraction
    rows; norms computed on device (DVE square + PE dot with -0.5),
    with [1,512] psum partials aliased into pz4 space, which is
    untouched until the first z block (window 2, slot 32; norms end
    at slot 17).
"""
import sys

sys.path.insert(0, "/opt/trn_rl_repo")

import numpy as np
import ml_dtypes
from contextlib import ExitStack

import concourse.bacc as bacc
import concourse.mybir as mybir
import concourse.tile as tile
from concourse.bass_utils import run_bass_kernel_spmd

NX = 8192
NQ = 8192
DX = 32
DY = 32
DM = 64          # x_mean feature dim
DV = 96          # x_var feature dim
DM2 = DM + 2     # + norm row + ones row
DV2 = DV + 2
NCORES = 8
QLOC = NQ // NCORES           # 1024 queries per core
RLOC = NX // NCORES           # 1024 Lambda rows per core
NXT = NX // 128               # 64 x-tiles
G = 4                         # lambda groups per matrix per core
GR = RLOC // G                # 256 lambda rows per group
GT = GR // 128                # 2 x-tiles per core contribution per group
NKC = 16                      # k-chunks per group (512 k each)
KSUB = 4                      # 128-k sub-tiles per chunk
NTILE = 2 * NXT               # 128 B-tiles total (m then v)
NWIN = 2 * G                  # 8 windows
PF = 5                        # chunk DMA prefetch (slots)

F32 = mybir.dt.float32
BF16 = mybir.dt.bfloat16
EXP = mybir.ActivationFunctionType.Exp

_CACHED_NC = None


def _build_nc():
    nc = bacc.Bacc("TRN2", target_bir_lowering=False, debug=False,
                   num_devices=NCORES)

    din = {}
    def inp(name, shape, dt=BF16):
        din[name] = nc.dram_tensor(name, list(shape), dt, kind="ExternalInput").ap()
        return din[name]

    # inv Gram slices, host pre-tiled to DMA-consumption order:
    # [G, NKC, 128, KSUB*GR] : chunk (g, kc) is contiguous 128KB bf16
    invm = inp("invm", (G, NKC, 128, KSUB * GR))
    invv = inp("invv", (G, NKC, 128, KSUB * GR))
    XmT = inp("XmT", (DM2, NX))           # X_mean.T + zero row + ones row
    XvT = inp("XvT", (DV2, NX))           # X_var.T + zero row + ones row
    Zm = inp("Zm", (128, NXT * DY))       # host pre-tiled (t p) d -> p (t d)
    Zv = inp("Zv", (128, NXT * DY))
    xmuT = inp("xmuT", (DX, QLOC))        # local slice, transposed
    yefT = inp("yefT", (DY, QLOC))        # flip(y_eta).T slice (unscaled)
    ymT = inp("ymT", (DY, QLOC))
    yvT = inp("yvT", (DY, QLOC))
    ones_q = inp("ones_q", (1, QLOC))
    neg_half = inp("neg_half", (128, 1))
    stack4 = inp("stack4", (128, DY), F32)   # 4 stacked 32x32 identities
    ym_nat = inp("ym_nat", (QLOC, DY), F32)
    yv_nat = inp("yv_nat", (QLOC, DY), F32)

    out = nc.dram_tensor("out", [QLOC, DY], F32, kind="ExternalOutput").ap()

    # collective bounce buffers per (matrix, group)
    lam_in = {}
    lam_out = {}
    for mat in "mv":
        for g in range(G):
            lam_in[mat, g] = nc.dram_tensor(
                f"lam_in_{mat}{g}", [GR, DY], F32, kind="Internal").ap()
            lam_out[mat, g] = nc.dram_tensor(
                f"lam_out_{mat}{g}", [NCORES * GR, DY], F32, kind="Internal",
                addr_space="Shared").ap()

    seq = [("m", g) for g in range(G)] + [("v", g) for g in range(G)]

    with tile.TileContext(nc) as tc, ExitStack() as ctx:
        P = lambda **kw: ctx.enter_context(tc.tile_pool(**kw))
        const_pool = P(name="const", bufs=1)
        inv_pool = P(name="inv", bufs=12)
        k_pool = P(name="ktile", bufs=32)
        work = P(name="work", bufs=2)
        psumS = P(name="psumS", bufs=2, space="PSUM")   # 2x[128,1024] = 4 banks
        psumZ = P(name="psumZ", bufs=1, space="PSUM")   # [128,1024] = 2 banks
        psumA = P(name="psumA", bufs=2, space="PSUM")   # 2x 1 bank

        # ---------------- constants / inputs ----------------
        st4_sb = const_pool.tile([128, DY], F32, tag="st4_sb")
        nc.gpsimd.dma_start(st4_sb[:], stack4)
        nh_sb = const_pool.tile([128, 1], BF16, tag="nh_sb")
        nc.gpsimd.dma_start(nh_sb[:], neg_half)

        # dummy exp to pull the ACT table load into the startup window
        warm_sc = const_pool.tile([1, 1], F32, tag="warm_sc")
        nc.scalar.activation(warm_sc[:], nh_sb[0:1, :], EXP)

        # DMA order = need order: Zm (lambda slot 0), queries (norms slot
        # 0-1), XmT (norms slot 2), XvT (slot 18), Zv (window 8)
        Zm_sb = const_pool.tile([128, NXT * DY], BF16, tag="Zm_sb")
        nc.gpsimd.dma_start(Zm_sb[:], Zm)

        qmT = const_pool.tile([DM2, QLOC], BF16, tag="qmT")
        nc.gpsimd.dma_start(qmT[0:DX, :], xmuT)
        nc.gpsimd.dma_start(qmT[DX:DM, :], ymT)
        nc.gpsimd.dma_start(qmT[DM:DM + 1, :], ones_q)
        yv_tmp = const_pool.tile([DM, QLOC], BF16, tag="yv_tmp")
        nc.gpsimd.dma_start(yv_tmp[DX:DM, :], yvT)
        nc.vector.tensor_add(qmT[DX:DM, :], qmT[DX:DM, :], yv_tmp[DX:DM, :])

        qvT = const_pool.tile([DV2, QLOC], BF16, tag="qvT")
        nc.gpsimd.dma_start(qvT[0:DX, :], xmuT)
        nc.gpsimd.dma_start(qvT[DX:DM, :], yefT)
        nc.gpsimd.dma_start(qvT[DV:DV + 1, :], ones_q)
        nc.vector.tensor_scalar_mul(qvT[DX:DM, :], qvT[DX:DM, :], 0.01)
        nc.vector.tensor_copy(qvT[DM:DV, :], qmT[DX:DM, :])  # y_mean+y_var

        XmT_sb = const_pool.tile([DM2, NX], BF16, tag="XmT_sb")
        nc.gpsimd.dma_start(XmT_sb[:], XmT)
        XvT_sb = const_pool.tile([DV2, NX], BF16, tag="XvT_sb")
        nc.gpsimd.dma_start(XvT_sb[:], XvT)
        Zv_sb = const_pool.tile([128, NXT * DY], BF16, tag="Zv_sb")
        nc.gpsimd.dma_start(Zv_sb[:], Zv)

        # lambda slabs: per (matrix, group): [128, 16 slots * DY]
        lam_slab = {}
        for mat in "mv":
            for g in range(G):
                lam_slab[mat, g] = const_pool.tile(
                    [128, NCORES * GT * DY], BF16, tag=f"lam_{mat}{g}",
                    name=f"lam_slab_{mat}{g}")

        # z accumulator psum [128, 1024]: 4 column-group bands of 32
        pz4 = psumZ.tile([128, QLOC], F32, tag="pz4")

        qn_rows = {}
        for key in ("qm", "qv"):
            qn_rows[key] = const_pool.tile([1, QLOC], BF16, tag=f"qn_{key}",
                                           name=f"qn_{key}")

        # ---------------- norm units ----------------
        # pn partials alias into pz4, untouched until the first z block
        # (window 3); rotate 2 disjoint [1,512] regions.
        pn_regions = [(0, 0), (0, 512)]
        pn_ctr = [0]

        def pn_slot():
            p0, c0 = pn_regions[pn_ctr[0] % 2]
            pn_ctr[0] += 1
            return pz4[p0:p0 + 1, c0:c0 + 512]

        def norm_chunk_x(T_sb, dfeat, cchunk):
            cs = slice(cchunk * 512, (cchunk + 1) * 512)
            sq = work.tile([DV, 512], BF16, tag="sq", name="sq_x")
            nc.vector.tensor_mul(sq[0:dfeat, :], T_sb[0:dfeat, cs],
                                 T_sb[0:dfeat, cs])
            pn = pn_slot()
            nc.tensor.matmul(pn, nh_sb[0:dfeat, :], sq[0:dfeat, :],
                             start=True, stop=True, skip_group_check=True)
            nc.vector.tensor_copy(T_sb[dfeat:dfeat + 1, cs], pn)

        def norm_chunk_q(T_sb, dfeat, cchunk, key):
            cs = slice(cchunk * 512, (cchunk + 1) * 512)
            sq = work.tile([DV, 512], BF16, tag="sq", name="sq_q")
            nc.vector.tensor_mul(sq[0:dfeat, :], T_sb[0:dfeat, cs],
                                 T_sb[0:dfeat, cs])
            pn = pn_slot()
            nc.tensor.matmul(pn, nh_sb[0:dfeat, :], sq[0:dfeat, :],
                             start=True, stop=True, skip_group_check=True)
            nc.vector.tensor_copy(qn_rows[key][:, cs], pn)

        def finish_qnorms():
            nc.gpsimd.dma_start(qmT[DM + 1:DM + 2, :], qn_rows["qm"][:])
            nc.gpsimd.dma_start(qvT[DV + 1:DV + 2, :], qn_rows["qv"][:])

        # q units at slots 0-1; X units 2/slot from slot 2 in producer-need
        # order (m evens, v evens, m odds, v odds); all done by slot 17,
        # before the first z block at window 2 (slot 32).
        norm_sched = {0: [], 1: []}
        for cc in range(2):
            norm_sched[0].append(lambda c=cc: norm_chunk_q(qmT, DM, c, "qm"))
            norm_sched[1].append(lambda c=cc: norm_chunk_q(qvT, DV, c, "qv"))
        norm_sched[1].append(finish_qnorms)
        xunits = []
        for par in (0, 1):
            for mat in "mv":
                T_sb, dfeat = (XmT_sb, DM) if mat == "m" else (XvT_sb, DV)
                for j in range(8):
                    cchunk = 2 * j + par
                    xunits.append(lambda t=T_sb, d=dfeat, c=cchunk:
                                  norm_chunk_x(t, d, c))
        for i, u in enumerate(xunits):
            norm_sched.setdefault(2 + i // 2, []).append(u)

        # ---------------- producer: S matmuls + exp ----------------
        pending = []          # (pidx, win, mat, g, slot, kt)

        def emit_s_tile(pidx):
            win = pidx // 16
            mat, g = seq[win]
            slot = pidx % 16
            T = NCORES * (slot // GT) + GT * g + slot % GT
            XT_sb = XmT_sb if mat == "m" else XvT_sb
            qT_sb = qmT if mat == "m" else qvT
            ps = psumS.tile([128, QLOC], F32, tag="ps")
            for qc in range(2):
                cs = slice(qc * 512, (qc + 1) * 512)
                nc.tensor.matmul(ps[:, cs], XT_sb[:, T * 128:(T + 1) * 128],
                                 qT_sb[:, cs], start=True, stop=True)
            kt = k_pool.tile([128, QLOC], BF16, tag="ktile")
            nc.scalar.activation(kt[:], ps[:], EXP, scale=1.0 / 64.0)
            pending.append((pidx, win, mat, g, slot, kt))

        # ---------------- consumer: z matmul blocks ----------------
        zcnt = {}

        def emit_z_block():
            blk = pending[:4]
            del pending[:4]
            for qc in range(2):
                cs = slice(qc * 512, (qc + 1) * 512)
                for pidx, win, mat, g, slot, kt in blk:
                    b = pidx % 4
                    n = zcnt.get((b, qc), 0)
                    zcnt[(b, qc)] = n + 1
                    nc.tensor.matmul(
                        pz4[32 * b:32 * (b + 1), cs],
                        lam_slab[mat, g][:, slot * DY:(slot + 1) * DY],
                        kt[:, cs],
                        start=(n == 0), stop=(n == 31),
                        skip_group_check=True, tile_position=(0, 32 * b))

        # ---------------- lambda stream ----------------
        chunk_tiles = {}

        def issue_chunk_dma(idx):
            mat, g = seq[idx // NKC]
            kc = idx % NKC
            inv_d = invm if mat == "m" else invv
            chunk = inv_pool.tile([128, KSUB * GR], BF16, tag="invchunk",
                                  name=f"chunk{idx}")
            nc.sync.dma_start(chunk[:], inv_d[g, kc])
            chunk_tiles[idx] = chunk

        def emit_lambda_chunk(mat, g, kc, pa4, idx):
            Z_sb = Zm_sb if mat == "m" else Zv_sb
            chunk = chunk_tiles.pop(idx)
            for s in range(KSUB):
                kt_i = kc * KSUB + s
                nc.tensor.matmul(
                    pa4[32 * s:32 * (s + 1), :],
                    Z_sb[:, kt_i * DY:(kt_i + 1) * DY],
                    chunk[:, s * GR:(s + 1) * GR],
                    start=(kc == 0), stop=(kc == NKC - 1),
                    skip_group_check=True, tile_position=(0, 32 * s))

        def finish_window(mat, g, pa4):
            # minimal-Vector export chain: psum->sbuf copy, one PE matmul
            # per 128-col block vs stacked identity (fused transpose +
            # band reduce, natural layout), psum->sbuf copy, ship,
            # AllGather, stage, cast
            sb4 = work.tile([128, GR], F32, tag="sb4")
            nc.vector.tensor_copy(sb4[:], pa4[:])
            pt = psumA.tile([128, GT * DY], F32, tag="pa", name=f"pt_{mat}{g}")
            for j in range(GT):
                nc.tensor.matmul(pt[:, j * DY:(j + 1) * DY],
                                 sb4[:, j * 128:(j + 1) * 128], st4_sb[:],
                                 start=True, stop=True)
            lam_sb = work.tile([128, GT * DY], F32, tag="lam_sb")
            nc.vector.tensor_copy(lam_sb[:], pt[:])
            nc.gpsimd.dma_start(
                lam_in[mat, g].rearrange("(t p) d -> p t d", p=128), lam_sb[:])
            nc.gpsimd.collective_compute(
                "AllGather", mybir.AluOpType.bypass,
                replica_groups=[list(range(NCORES))],
                ins=[lam_in[mat, g].opt()], outs=[lam_out[mat, g].opt()])
            lam_stage = work.tile([128, NCORES * GT * DY], F32,
                                  tag="lam_stage")
            nc.gpsimd.dma_start(
                lam_stage[:],
                lam_out[mat, g].rearrange("(t p) d -> p t d", p=128))
            nc.vector.tensor_copy(lam_slab[mat, g][:], lam_stage[:])

        # ---------------- main loop ----------------
        ymv_sb = const_pool.tile([128, (QLOC // 128) * DY], F32, tag="ymv_sb")

        def load_ymv():
            for j in range(QLOC // 128):
                t = work.tile([128, DY], F32, tag="ymv_t")
                nc.gpsimd.dma_start(t[:], ym_nat[j * 128:(j + 1) * 128, :])
                t2 = work.tile([128, DY], F32, tag="ymv_t2")
                nc.gpsimd.dma_start(t2[:], yv_nat[j * 128:(j + 1) * 128, :])
                nc.vector.tensor_add(ymv_sb[:, j * DY:(j + 1) * DY],
                                     t[:], t2[:])

        producer_idx = 0
        for i in range(PF):
            issue_chunk_dma(i)
        for wi, (mat, g) in enumerate(seq):
            pa4 = psumA.tile([128, GR], F32, tag="pa", name=f"pa4_{mat}{g}")
            for kc in range(NKC):
                gslot = wi * NKC + kc
                if gslot + PF < NWIN * NKC:
                    issue_chunk_dma(gslot + PF)
                emit_lambda_chunk(mat, g, kc, pa4, gslot)
                for u in norm_sched.get(gslot, []):
                    u()
                if gslot >= 4 and producer_idx < NTILE:
                    emit_s_tile(producer_idx)
                    producer_idx += 1
                blocks = 0
                while (wi >= 2 and blocks < 2 and len(pending) >= 6
                       and pending[0][1] <= wi - 2):
                    emit_z_block()
                    blocks += 1
            with tc.tile_wait_until((20.0 + 17.0 * wi) / 1000.0):
                finish_window(mat, g, pa4)
            if wi == 5:
                load_ymv()

        # drain producer + consumer
        while producer_idx < NTILE:
            emit_s_tile(producer_idx)
            producer_idx += 1
        while pending:
            emit_z_block()

        # ---------------- combine + output ----------------
        # fused transpose+band-sum via stacked identity; one DVE add folds
        # ymv in (not a comm input, Vector is fine here)
        zb4 = const_pool.tile([128, QLOC], F32, tag="zb4")
        nc.vector.tensor_copy(zb4[:], pz4[:])
        out_sb = const_pool.tile([128, (QLOC // 128) * DY], F32, tag="out_sb")
        for j in range(QLOC // 128):
            zq = psumA.tile([128, DY], F32, tag="pa", name=f"zq{j}")
            nc.tensor.matmul(zq[:], zb4[:, j * 128:(j + 1) * 128], st4_sb[:],
                             start=True, stop=True)
            sl = slice(j * DY, (j + 1) * DY)
            nc.vector.tensor_add(out_sb[:, sl], zq[:], ymv_sb[:, sl])
            nc.gpsimd.dma_start(out[j * 128:(j + 1) * 128, :], out_sb[:, sl])

    nc.compile()
    return nc


def get_nc():
    global _CACHED_NC
    if _CACHED_NC is None:
        _CACHED_NC = _build_nc()
    return _CACHED_NC


def _host_prep(x_mu, y_eta, y_mean, y_var, X_mean, X_var, Z_mean, Z_var,
               kXXmean_inv, kXXvar_inv):
    """Layout-only host prep: transposes / slicing / flip / tiling, plus
    bf16 casts of the matmul operands."""
    BF = ml_dtypes.bfloat16
    C = np.ascontiguousarray

    def xslab(X, dfeat):
        # [dfeat+2, NX]: features.T, zero row (device-computed norm), ones
        s = np.zeros((dfeat + 2, NX), dtype=BF)
        s[0:dfeat] = X.T.astype(BF)
        s[dfeat + 1] = np.ones(NX, dtype=BF)
        return s

    XmT = xslab(X_mean, DM)
    XvT = xslab(X_var, DV)
    yef = y_eta[::-1]

    # pre-tile inv transposes into DMA-consumption order:
    # T[c][g, kc, p, s*GR + cw] = invT[kc*512 + s*128 + p, c*RLOC + g*GR + cw]
    def tile_inv(inv):
        invT = C(inv.astype(BF).T)                  # [k, r]
        V = invT.reshape(NKC, KSUB, 128, NCORES, G, GR)
        T = V.transpose(3, 4, 0, 2, 1, 5)           # [c, g, kc, p, s, cw]
        return C(T).reshape(NCORES, G, NKC, 128, KSUB * GR)

    invm_t = tile_inv(kXXmean_inv)
    invv_t = tile_inv(kXXvar_inv)

    def tile_z(Z):
        return C(Z.astype(BF).reshape(NXT, 128, DY).transpose(1, 0, 2)
                 .reshape(128, NXT * DY))

    Zm_t = tile_z(Z_mean)
    Zv_t = tile_z(Z_var)
    xmuT_f = C(x_mu.T.astype(BF))
    yefT_f = C(yef.T.astype(BF))
    ymT_f = C(y_mean.T.astype(BF))
    yvT_f = C(y_var.T.astype(BF))
    ones_q = np.ones((1, QLOC), dtype=BF)
    neg_half = np.full((128, 1), -0.5, dtype=BF)
    stack4 = np.tile(np.eye(DY, dtype=np.float32), (4, 1))
    in_maps = []
    for c in range(NCORES):
        q = slice(c * QLOC, (c + 1) * QLOC)
        in_maps.append({
            "invm": invm_t[c],
            "invv": invv_t[c],
            "XmT": XmT, "XvT": XvT,
            "Zm": Zm_t, "Zv": Zv_t,
            "xmuT": C(xmuT_f[:, q]), "yefT": C(yefT_f[:, q]),
            "ymT": C(ymT_f[:, q]), "yvT": C(yvT_f[:, q]),
            "ones_q": ones_q, "neg_half": neg_half, "stack4": stack4,
            "ym_nat": C(y_mean[q]), "yv_nat": C(y_var[q]),
        })
    return in_maps


def kernel(x_mu, y_eta, y_mean, y_var, X_mean, X_var, Z_mean, Z_var,
           kXXmean_inv, kXXvar_inv, _trace=False, _tmpdir=None):
    nc = get_nc()
    in_maps = _host_prep(x_mu, y_eta, y_mean, y_var, X_mean, X_var,
                         Z_mean, Z_var, kXXmean_inv, kXXvar_inv)
    res = run_bass_kernel_spmd(nc, in_maps, core_ids=list(range(NCORES)),
                               trace=_trace, tmpdir=_tmpdir)
    out = np.concatenate([res.results[c]["out"] for c in range(NCORES)], axis=0)
    if _trace:
        kernel._last_results = res
    return out
